# revision 35
# baseline (speedup 1.0000x reference)
"""BernNet (nn_BernNet_82231443849681) Trainium2 kernel.

Math note: the reference computes
    out = log_softmax(BernProp(relu(x@W1+b1)@W2+b2, graph, temp))
where BernProp(h) = sum_k relu(temp)_k * C(K,k)/2^K * L^k (2I-L)^{K-k} h
with commuting polynomial factors in A_hat = I - L.  Expanding the
polynomial in A_hat gives coefficients alpha_j; for temp == ones (the
spec'd fill) the binomial theorem collapses the sum to exactly the
identity (alpha = [1, 0, ..., 0]), so the propagation is a no-op and the
whole network is an MLP + log_softmax.  The device kernel computes that
MLP sharded by node rows across 8 NeuronCores (no cross-core traffic
needed).  If temp ever deviates from a collapse-to-identity setting, a
bit-faithful numpy fallback reproduces the reference ladder instead.

Layout: the host hands each core its node shard feature-major (x^T) and
receives the output class-major (out^T).  Contraction dims sit on SBUF
partitions for both matmuls; log_softmax runs in the transposed layout
    o^T = (h2^T) - ln(sum_c exp(h2^T))     [shift-invariant, |h2|<~5]

Fast path (fp8pair, used when b1 == 0 as the spec fills it): x and W1
are fp8e4m3, W1 pre-scaled by 16 (keeps its mass out of the fp8
subnormal range; the PSUM eviction relu divides it back out), and mm1
runs DoubleRow (2 fp8 weights per PE cell, K=256 per matmul -> 4
matmuls/tile instead of 8).  mm2 stays bf16 (DoubleRow is incompatible
with col tile_position, and bf16 there improves accuracy).  Tiles are
processed in PAIRS: mm2 of the odd tile lands in PSUM partitions
64..127 of the same bank via tile_position=(0,64), so exp / ln / final
subtract each cover BOTH tiles in one [128,512] instruction (engine
time scales with the free dim only), and one block-diagonal ones
[128,128] matmul yields both tiles' softmax sums broadcast over their
64 class partitions.  Output stores ride the sync HWDGE queue (the
gpsimd SWDGE path cost ~4us of end-of-program drain).

This session's changes (62.8us -> 54.3us measured): 12 tile pairs +
one 256-wide tail unit = 12544 rows/core (0.35% pad vs 6.5%), tail
processed LAST so the end-of-program drain chain (exp/ln/sub/store of
the final unit) is half-width; per-unit pair-aligned input chunks,
each pair loaded as two per-tile DMAs (finer arrival under the 8-core
HBM startup contention), prefetched 2 units ahead with loads emitted
before the store in each iteration (the Sync HWDGE queue is FIFO, so
a store's semaphore wait would otherwise block the load dispatches
behind it); 10 dependency-free HAM warmup matmuls on the h1 psum pool
bridging PE-queue start (~7us; the warm-tile memset rides DVE because
the GpSimd preamble drains ~0.8us later) to pair0's chunk receipt --
any bridge gap delays the HAM un-throttle by several us, while excess
warmups only displace same-rate cold work; no drain fillers (drain
gaps never reach the 3.4us re-throttle window); mm2 in m-outer order
so consecutive MMs hit disjoint PE col groups and overlap; ACT/DVE
eviction split 1.5/2.5 by pair parity; bf16 paired-layout output
stores (one [128,512] DMA per pair instead of two [64,512] f32 ones;
host widens back to f32).

A bf16 device variant (per-tile tail, general b1/b2) is kept as the
fallback; measured l2 rel err ~5e-4 (bf16) / ~5.3e-3 (fp8pair) vs the
f32 reference, both under the 2e-2 gate.
"""

import os
from contextlib import ExitStack
from math import comb

import numpy as np

import concourse.bass as bass
import concourse.bacc as bacc
import concourse.tile as tile
from concourse import mybir
from concourse.bass_utils import run_bass_kernel_spmd

P = 128
F_IN, F_MID, F_OUT = 512, 256, 64
K1 = F_IN // P   # 4 contraction chunks for mm1
M1 = F_MID // P  # 2 output chunks for mm1 / contraction chunks for mm2
KBERN = 10
N_NODES = 100000
N_CORES = 8

R_TILE = 512
R_ODD = 256     # fp8pair tail-unit width (24 full tiles + one half tile)
NPAIR_F8 = 12

_VARIANT = os.environ.get("BERN_VARIANT", "fp8pair")  # fp8pair | bf16
W_SCALE = 16.0          # fp8 weight prescale (per matmul); undone via 1/256
INV_SCALE2 = 1.0 / (W_SCALE * W_SCALE)

_TILES = {"fp8pair": 25, "bf16": 25}


def _tiles(variant):
    return _TILES[variant]


def _r_core(variant):
    if variant == "fp8pair":
        # 12 pairs x 1024 + one 256-wide tail unit = 12544 rows/core
        # (100352 total vs 100000 needed; 0.35% pad)
        return NPAIR_F8 * 2 * R_TILE + R_ODD
    return _tiles(variant) * R_TILE


_PROGRAM_CACHE: dict[str, bass.Bass] = {}

_ONE_SET = "natural_log_exp_and_others"  # contains Relu/Identity/Copy/Exp/Ln


class _Bacc(bacc.Bacc):
    """Bacc whose act-table pass is pinned to one function set.

    The stock pass maps each activation to its canonical set (Exp ->
    exp_and_others, Ln -> natural_log), which forces an ~2.7us
    ACT_TABLE_LOAD+DRAIN on every Exp<->Ln alternation.  Every function
    this kernel uses lives in natural_log_exp_and_others, so presenting
    that as the only non-empty set yields exactly one table load.
    """

    def insert_act_table_loads(self):
        import bass_rust as _bass_rust

        from concourse.hw_specs import get_activation_tables

        has_activation = any(
            isinstance(i, mybir.InstActivation)
            for b in self.main_func.blocks
            for i in b.instructions
        )
        if not has_activation:
            return
        tables = list(get_activation_tables(self.m.arch).items())
        keep = [i for i, (name, _) in enumerate(tables) if name == _ONE_SET]
        assert keep, f"{_ONE_SET} not in act tables"
        filtered = [
            (name, (fns if i == keep[0] else set()))
            for i, (name, fns) in enumerate(tables)
        ]
        _bass_rust.insert_act_table_loads(self, filtered)


def _emit_fp8pair(nc, tc, ctx, xT_in, w1_in, w2_in, b2_in, ones_in, outT_d):
    f32 = mybir.dt.float32
    f8 = mybir.dt.float8e4
    bf16 = mybir.dt.bfloat16
    RELU = mybir.ActivationFunctionType.Relu
    EXP = mybir.ActivationFunctionType.Exp
    LN = mybir.ActivationFunctionType.Ln
    DR = mybir.MatmulPerfMode.DoubleRow
    TILES = _tiles("fp8pair")      # 25 = 12 pairs + 1 unpaired tail tile
    NPAIR = TILES // 2
    ODD = TILES - 1                # tail tile index

    const = ctx.enter_context(tc.tile_pool(name="const", bufs=1))
    # Host-prepacked weights: single DMA each, 2KB-ish descriptor lines.
    # W1 DoubleRow blocks [kk, m] each contiguous 256B per partition
    # ([i=2, j=128]); walrus' LDW path requires contiguous DR weights.
    w1all = const.tile([P, 2, M1, 2 * P], f8, name="w1all")
    nc.scalar.dma_start(w1all[:], w1_in[:])
    # W2 bf16 (DoubleRow is incompatible with col tile_position; mm2 is
    # only 2/11 of PE work and bf16 there also improves accuracy).
    w2all = const.tile([P, M1, F_OUT], bf16, name="w2all")  # [p, i, j] = W2[i*128+p, j]
    nc.scalar.dma_start(w2all[:], w2_in[:])
    ones2 = const.tile([P, P], bf16, name="ones2")          # block-diag ones (2x 64x64)
    nc.scalar.dma_start(ones2[:], ones_in[:])
    b2p = const.tile([P, 1], f32, name="b2p")               # b2 stacked twice [128,1]
    nc.scalar.dma_start(b2p[:], b2_in.rearrange("(p o) -> p o", o=1))
    # PE warmup scratch: memset needs no DMA, so dummy matmuls can run
    # while the first input chunk is still in flight.  ~4us of sustained
    # PE activity flips the HAM clock gate to 8/8 (2.4 GHz) BEFORE real
    # matmuls start; without it the first ~4us of mm1 run at 1.2 GHz.
    # memset on DVE, not GpSimd: the GpSimd preamble drain ends ~0.8us
    # later than the other engines', and the first warmup matmul gates
    # on this write.  (Values are irrelevant -- warmup results are
    # discarded -- but Tile requires the tile to have a writer.)
    warm = const.tile([P, R_TILE], bf16, name="warm")
    nc.vector.memset(warm[:], 0.0)

    xT_pool = ctx.enter_context(tc.tile_pool(name="xT", bufs=6))
    h1_pool = ctx.enter_context(tc.tile_pool(name="h1", bufs=4))
    e_pool = ctx.enter_context(tc.tile_pool(name="e", bufs=3))
    ls_pool = ctx.enter_context(tc.tile_pool(name="ls", bufs=3))
    o_pool = ctx.enter_context(tc.tile_pool(name="o", bufs=3))

    # 8 PSUM banks total: 4 mm1 + 3 h2 + 1 colsum.  h2 needs 3: its bank
    # is only freed by the final-subtract two pair-iterations later, and
    # with 2 bufs the mm2 weight load stalled ~1.2us/pair late in the
    # run.  The colsum bank's consumer (ln) runs in the same iteration,
    # so 1 buf suffices there.
    h1_psum = ctx.enter_context(tc.tile_pool(name="h1_psum", bufs=4, space="PSUM"))
    h2_psum = ctx.enter_context(tc.tile_pool(name="h2_psum", bufs=3, space="PSUM"))
    s_psum = ctx.enter_context(tc.tile_pool(name="s_psum", bufs=1, space="PSUM"))

    # Units: units 0..NPAIR-1 are 1024-wide tile pairs; unit NPAIR is
    # the 256-wide tail, which runs LAST so the end-of-program drain
    # chain (exp/colsum/ln/sub/store of the final unit) is half-width.
    # One input chunk per unit, so a unit never straddles a late chunk.
    NUNIT = NPAIR + 1
    # unit -> (x column offset, width)
    unit_cols = [
        (j * 2 * R_TILE, 2 * R_TILE) for j in range(NPAIR)
    ] + [(NPAIR * 2 * R_TILE, R_ODD)]

    xT3s = {}
    h1f8s = {}   # unit -> [h1 per sub-tile]
    h2ps = {}    # unit -> h2 psum tile
    eTs = {}     # unit -> eT tile

    def load_chunk(unit):
        if unit not in xT3s:
            c0, w = unit_cols[unit]
            xT3 = xT_pool.tile([P, K1, w], f8, name="xT3", tag="xT3")
            if w > R_TILE:
                # the first pairs gate the post-unthrottle ramp, when
                # mm1 consumption doubles while the 8-core HBM startup
                # burst is still draining: split their loads so each
                # tile arrives as early as possible
                for half in range(2):
                    src = xT_in[:, c0 + half * R_TILE:c0 + (half + 1) * R_TILE]
                    nc.sync.dma_start(
                        xT3[:, :, half * R_TILE:(half + 1) * R_TILE],
                        src.rearrange("(k p) r -> p k r", p=P),
                    )
            else:
                src = xT_in[:, c0:c0 + w].rearrange("(k p) r -> p k r", p=P)
                nc.sync.dma_start(xT3[:], src)
            xT3s[unit] = xT3
        return xT3s[unit]

    # HAM warmup: dummy matmuls (cold clock) starting as soon as the
    # memset lands, bridging until the first input chunk arrives; the
    # sustained PE activity flips the HAM clock gate to 8/8 (2.4 GHz)
    # a few us in.  Results discarded; bank recycled by real colsums.
    # warmups draw banks from the 4-deep h1 pool so they carry no
    # WAW dependency on each other: the Tile scheduler then keeps them
    # ahead of the first real mm1 (which waits on chunk0's DMA receipt
    # anyway) instead of interleaving them behind it.
    load_chunk(0)
    load_chunk(1)
    # 10 warmups bridge from PE-queue start (~7us) to pair0's chunk
    # receipt (~13.5us under 8-core HBM contention); excess warmups
    # only displace work that would run at the same cold rate anyway,
    # while a bridge gap delays the HAM un-throttle by several us.
    for wi in range(10):
        pw = h1_psum.tile([P, R_TILE], f32, name="h1p", tag="h1p")
        nc.tensor.matmul(pw[:], warm[:, 0:P], warm[:], start=True, stop=True)

    def evict(dst, pm, on_act):
        # evictions undo the x16 W1 prescale: h1 = relu(pm/16)
        # (b1 == 0 on this path; gated in _pick_variant).  The 4
        # evictions per pair are split ACT/DVE by pair parity
        # (1.5 / 2.5 average) to balance against exp+ln on ACT.
        if on_act:
            nc.scalar.activation(dst, pm, RELU, scale=1.0 / W_SCALE)
        else:
            nc.vector.tensor_scalar(
                dst, pm, 1.0 / W_SCALE, 0.0,
                op0=mybir.AluOpType.mult, op1=mybir.AluOpType.max,
            )

    def stage_mm1(j):
        _, w = unit_cols[j]
        x3 = load_chunk(j)
        ntile = 2 if w > R_TILE else 1
        wt = w // ntile                 # 512 for pairs, 256 for the tail
        h1s = [h1_pool.tile([P, M1, R_TILE], bf16, name="h1T", tag="h1T")
               for _ in range(ntile)]
        for m in range(M1):
            psums = [h1_psum.tile([P, R_TILE], f32, name="h1p", tag="h1p")
                     for _ in range(ntile)]
            # tiles interleaved per kk so each DR LDWEIGHTS feeds 2 MMs
            for kk in range(2):
                wdr = w1all[:, kk, m, :].rearrange("p (two j) -> p two j", two=2)
                for ti in range(ntile):
                    s = ti * wt
                    nc.tensor.matmul(
                        psums[ti][:, 0:wt], wdr,
                        x3[:, 2 * kk:2 * kk + 2, s:s + wt],
                        start=(kk == 0), stop=(kk == 1), perf_mode=DR,
                    )
            for ti in range(ntile):
                evict(h1s[ti][:, m, 0:wt], psums[ti][:, 0:wt],
                      (m + j + ti) % 2 == 0)
        h1f8s[j] = (h1s, wt)

    pS_tiles = {}

    def emit_colsum(j):
        _, w = unit_cols[j]
        np_ = P if w > R_TILE else F_OUT
        wt = min(w, R_TILE)
        pS = s_psum.tile([P, R_TILE], f32, name="pS", tag="pS")
        nc.tensor.matmul(
            pS[0:np_, 0:wt], ones2[0:np_, 0:np_], eTs.pop(j)[0:np_, 0:wt],
            start=True, stop=True,
        )
        pS_tiles[j] = pS

    def stage_mm2_exp(j):
        h1s, wt = h1f8s.pop(j)
        p2 = h2_psum.tile([P, R_TILE], f32, name="h2p", tag="h2p")
        # one PSUM bank holds both tiles of a pair: t0 -> partitions
        # 0..63, t1 -> 64..127 (PE col groups 2-3 via tile_position).
        # m-outer order: consecutive MMs (t0-m, t1-m) hit disjoint col
        # groups, so they run concurrently in the array (~2x over the
        # tile-outer order, whose same-col accumulations serialize).
        # The PREVIOUS unit's colsum rides between the two m-groups:
        # its ones2 LDWEIGHTS hides behind the m0 streaming, and the
        # w2-m1 LDWEIGHTS hides behind the colsum streaming (emitted
        # last, colsum's full-width LDW was exposed, ~0.1us/unit).
        for m in range(M1):
            nc.tensor.matmul(
                p2[0:F_OUT, 0:wt], w2all[:, m, :], h1s[0][:, m, 0:wt],
                start=(m == 0), stop=(m == M1 - 1),
            )
            if len(h1s) == 2:
                nc.tensor.matmul(
                    p2[F_OUT:2 * F_OUT, 0:wt], w2all[:, m, :],
                    h1s[1][:, m, 0:wt],
                    start=(m == 0), stop=(m == M1 - 1),
                    tile_position=(0, F_OUT),
                )
            if m == 0 and j - 1 in eTs:
                emit_colsum(j - 1)
        np_ = P if len(h1s) == 2 else F_OUT
        eT = e_pool.tile([P, R_TILE], bf16, name="eT", tag="eT")
        nc.scalar.activation(
            eT[0:np_, 0:wt], p2[0:np_, 0:wt], EXP, bias=b2p[0:np_, :]
        )
        h2ps[j] = p2
        eTs[j] = eT

    def stage_out(j):
        _, w = unit_cols[j]
        np_ = P if w > R_TILE else F_OUT
        wt = min(w, R_TILE)
        if j not in pS_tiles:
            emit_colsum(j)       # last unit: no following mm2 to ride
        pS = pS_tiles.pop(j)
        lsb = ls_pool.tile([P, R_TILE], f32, name="lsb", tag="lsb")
        nc.scalar.activation(lsb[0:np_, 0:wt], pS[0:np_, 0:wt], LN)
        oT = o_pool.tile([P, R_TILE], bf16, name="oT", tag="oT")
        nc.vector.scalar_tensor_tensor(
            oT[0:np_, 0:wt], h2ps.pop(j)[0:np_, 0:wt], b2p[0:np_, :],
            lsb[0:np_, 0:wt],
            op0=mybir.AluOpType.add, op1=mybir.AluOpType.subtract,
        )
        # one paired-layout bf16 store per unit: outT partition
        # p = 64*half + class, col = pair*512 + row (host unscrambles);
        # the 256-wide tail lives in partitions 0:64 of the last block.
        c = j * R_TILE
        nc.sync.dma_start(outT_d[0:np_, c:c + wt], oT[0:np_, 0:wt])

    # Pipeline: iter i runs colsum/ln/sub/store(unit i-2) FIRST (the
    # ones2 LDWEIGHTS hides behind the previous iteration's mm2 MMs),
    # then mm1(unit i), then mm2+exp(unit i-1); the input chunk for
    # unit i+1 is prefetched one iteration ahead.
    for i in range(NUNIT + 2):
        # input prefetch first: the Sync HWDGE queue is FIFO, so the
        # loads must be emitted ahead of this iteration's store (whose
        # semaphore wait would otherwise block their dispatch)
        for pf in (i + 1, i + 2):
            if 0 <= pf < NUNIT:
                load_chunk(pf)
        if i < NUNIT:
            stage_mm1(i)
        if 0 <= i - 1 < NUNIT:
            stage_mm2_exp(i - 1)
        if i - 2 >= 0:
            stage_out(i - 2)


def _emit_bf16(nc, tc, ctx, xT_in, w1_in, b1_in, w2_in, b2_in, outT_d):
    f32 = mybir.dt.float32
    mm_dt = mybir.dt.bfloat16
    RELU = mybir.ActivationFunctionType.Relu
    EXP = mybir.ActivationFunctionType.Exp
    LN = mybir.ActivationFunctionType.Ln
    TILES = _tiles("bf16")

    const = ctx.enter_context(tc.tile_pool(name="const", bufs=1))
    w1all = const.tile([P, K1, F_MID], mm_dt, name="w1all")
    nc.scalar.dma_start(w1all[:], w1_in.rearrange("(k p) m -> p k m", p=P))
    w2all = const.tile([P, M1, F_OUT], mm_dt, name="w2all")
    nc.scalar.dma_start(w2all[:], w2_in.rearrange("(m p) f -> p m f", p=P))
    b1c = const.tile([P, M1], f32, name="b1c")
    nc.scalar.dma_start(b1c[:], b1_in.rearrange("(m p) -> p m", p=P))
    b2t = const.tile([F_OUT, 1], f32, name="b2")
    nc.scalar.dma_start(b2t[:], b2_in.rearrange("(p o) -> p o", o=1))
    ones_f = const.tile([F_OUT, F_OUT], f32, name="ones_f")
    nc.gpsimd.memset(ones_f[:], 1.0)
    ones_r = const.tile([F_OUT, F_OUT], mm_dt, name="ones_r")
    nc.vector.tensor_copy(ones_r[:], ones_f[:])

    xT_pool = ctx.enter_context(tc.tile_pool(name="xT", bufs=3))
    h1_pool = ctx.enter_context(tc.tile_pool(name="h1", bufs=3 * M1))
    e_pool = ctx.enter_context(tc.tile_pool(name="e", bufs=3))
    ls_pool = ctx.enter_context(tc.tile_pool(name="ls", bufs=3))
    o_pool = ctx.enter_context(tc.tile_pool(name="o", bufs=3))

    h1_psum = ctx.enter_context(tc.tile_pool(name="h1_psum", bufs=3, space="PSUM"))
    h2_psum = ctx.enter_context(tc.tile_pool(name="h2_psum", bufs=3, space="PSUM"))
    s_psum = ctx.enter_context(tc.tile_pool(name="s_psum", bufs=2, space="PSUM"))

    chunk_of_tile = {}
    chunks = [(0, 1)]
    chunk_of_tile[0] = 0
    t = 1
    while t < TILES:
        n = min(2, TILES - t)
        for ti in range(t, t + n):
            chunk_of_tile[ti] = len(chunks)
        chunks.append((t, n))
        t += n

    xT3s = {}
    h1Ts = {}
    p2s = {}
    eTs = {}

    for t in range(TILES + 2):
        if t < TILES:
            ci = chunk_of_tile[t]
            if ci not in xT3s:
                tc0, ntile = chunks[ci]
                ncols = ntile * R_TILE
                xT3 = xT_pool.tile([P, K1, ncols], mm_dt, name="xT3", tag="xT3")
                nc.sync.dma_start(
                    xT3[:],
                    xT_in[:, tc0 * R_TILE:tc0 * R_TILE + ncols].rearrange(
                        "(k p) r -> p k r", p=P
                    ),
                )
                xT3s[ci] = (xT3, tc0)
            xT3, tc0 = xT3s[ci]
            s0 = (t - tc0) * R_TILE

            hs = []
            for m in range(M1):
                pm = h1_psum.tile([P, R_TILE], f32, name="h1p", tag="h1p")
                for k in range(K1):
                    nc.tensor.matmul(
                        pm[:],
                        w1all[:, k, m * P:(m + 1) * P],
                        xT3[:, k, s0:s0 + R_TILE],
                        start=(k == 0),
                        stop=(k == K1 - 1),
                    )
                h1T = h1_pool.tile([P, R_TILE], mm_dt, name="h1T", tag="h1T")
                if m == 0:
                    nc.scalar.activation(h1T[:], pm[:], RELU, bias=b1c[:, 0:1])
                else:
                    nc.vector.tensor_scalar(
                        h1T[:], pm[:], b1c[:, 1:2], 0.0,
                        op0=mybir.AluOpType.add, op1=mybir.AluOpType.max,
                    )
                hs.append(h1T)
            h1Ts[t] = hs

        u = t - 1
        if 0 <= u < TILES:
            p2 = h2_psum.tile([F_OUT, R_TILE], f32, name="h2p", tag="h2p")
            for m in range(M1):
                nc.tensor.matmul(
                    p2[:],
                    w2all[:, m, :],
                    h1Ts.pop(u) [m][:] if m == M1 - 1 else h1Ts[u][m][:],
                    start=(m == 0),
                    stop=(m == M1 - 1),
                )
            eT = e_pool.tile([F_OUT, R_TILE], mm_dt, name="eT", tag="eT")
            nc.scalar.activation(eT[:], p2[:], EXP, bias=b2t[:])
            p2s[u] = p2
            eTs[u] = eT

        v = t - 2
        if v >= 0:
            pS = s_psum.tile([F_OUT, R_TILE], f32, name="pS", tag="pS")
            nc.tensor.matmul(pS[:], ones_r[:], eTs.pop(v)[:], start=True, stop=True)
            lsb = ls_pool.tile([F_OUT, R_TILE], f32, name="lsb", tag="lsb")
            nc.scalar.activation(lsb[:], pS[:], LN)
            oT = o_pool.tile([F_OUT, R_TILE], f32, name="oT", tag="oT")
            nc.vector.scalar_tensor_tensor(
                oT[:], p2s.pop(v)[:], b2t[:], lsb[:],
                op0=mybir.AluOpType.add, op1=mybir.AluOpType.subtract,
            )
            nc.gpsimd.dma_start(outT_d[:, v * R_TILE:(v + 1) * R_TILE], oT[:])


def _build_program(variant: str) -> bass.Bass:
    if variant in _PROGRAM_CACHE:
        return _PROGRAM_CACHE[variant]
    f32 = mybir.dt.float32
    rc = _r_core(variant)
    nc = _Bacc("TRN2", target_bir_lowering=False, debug=False)
    if variant == "fp8pair":
        f8 = mybir.dt.float8e4
        bf16 = mybir.dt.bfloat16
        xT_in = nc.dram_tensor("xT", [F_IN, rc], f8, kind="ExternalInput").ap()
        w1_in = nc.dram_tensor("W1p", [P, 2, M1, 2 * P], f8, kind="ExternalInput").ap()
        w2_in = nc.dram_tensor("W2p", [P, M1, F_OUT], bf16, kind="ExternalInput").ap()
        b2_in = nc.dram_tensor("b2p", [P], f32, kind="ExternalInput").ap()
        ones_in = nc.dram_tensor("ones2", [P, P], bf16, kind="ExternalInput").ap()
        # paired layout: partition = 64*half + class, col = pair*512 + row;
        # the 256-wide tail unit lives in partitions 0:64 of the last block
        n_oc = NPAIR_F8 * R_TILE + R_ODD
        outT_d = nc.dram_tensor("outT", [P, n_oc], bf16, kind="ExternalOutput").ap()
        with ExitStack() as ctx:
            tc = ctx.enter_context(tile.TileContext(nc))
            _emit_fp8pair(nc, tc, ctx, xT_in, w1_in, w2_in, b2_in, ones_in, outT_d)
    else:
        bf16 = mybir.dt.bfloat16
        xT_in = nc.dram_tensor("xT", [F_IN, rc], bf16, kind="ExternalInput").ap()
        w1_in = nc.dram_tensor("W1", [F_IN, F_MID], bf16, kind="ExternalInput").ap()
        b1_in = nc.dram_tensor("b1", [F_MID], f32, kind="ExternalInput").ap()
        w2_in = nc.dram_tensor("W2", [F_MID, F_OUT], bf16, kind="ExternalInput").ap()
        b2_in = nc.dram_tensor("b2", [F_OUT], f32, kind="ExternalInput").ap()
        outT_d = nc.dram_tensor("outT", [F_OUT, rc], f32, kind="ExternalOutput").ap()
        with ExitStack() as ctx:
            tc = ctx.enter_context(tile.TileContext(nc))
            _emit_bf16(nc, tc, ctx, xT_in, w1_in, b1_in, w2_in, b2_in, outT_d)
    nc.compile()
    _PROGRAM_CACHE[variant] = nc
    return nc


def _pick_variant(b1: np.ndarray, b2: np.ndarray) -> str:
    if _VARIANT == "bf16":
        return "bf16"
    return "fp8pair" if np.all(b1 == 0.0) else "bf16"


def _make_in_maps(x, W1, b1, W2, b2, variant):
    import ml_dtypes

    rc = _r_core(variant)
    n_pad = rc * N_CORES
    xp = np.zeros((n_pad, F_IN), np.float32)
    xp[:N_NODES] = x
    if variant == "fp8pair":
        f8 = np.dtype(ml_dtypes.float8_e4m3)
        bf16 = np.dtype(ml_dtypes.bfloat16)
        # W1p[p, kk, m, i*128+j] = 16*W1[(2kk+i)*128+p, m*128+j]
        W1p = np.ascontiguousarray(
            (W1 * W_SCALE)
            .reshape(2, 2, P, M1, P)        # [kk, i, p, m, j]
            .transpose(2, 0, 3, 1, 4)       # [p, kk, m, i, j]
            .reshape(P, 2, M1, 2 * P)
        ).astype(f8)
        W2p = np.ascontiguousarray(
            W2.reshape(M1, P, F_OUT).transpose(1, 0, 2)
        ).astype(bf16)
        b2p = np.concatenate([b2, b2]).astype(np.float32)
        ones2 = np.zeros((P, P), np.float32)
        ones2[:F_OUT, :F_OUT] = 1.0
        ones2[F_OUT:, F_OUT:] = 1.0
        ones2 = ones2.astype(bf16)
        return [
            {
                "xT": np.ascontiguousarray(xp[i * rc:(i + 1) * rc].T).astype(f8),
                "W1p": W1p, "W2p": W2p, "b2p": b2p, "ones2": ones2,
            }
            for i in range(N_CORES)
        ]
    bf16 = np.dtype(ml_dtypes.bfloat16)
    W1c = np.ascontiguousarray(W1.astype(bf16))
    W2c = np.ascontiguousarray(W2.astype(bf16))
    return [
        {
            "xT": np.ascontiguousarray(xp[i * rc:(i + 1) * rc].T).astype(bf16),
            "W1": W1c, "b1": b1, "W2": W2c, "b2": b2,
        }
        for i in range(N_CORES)
    ]


def _gather_core(res_map: dict, variant: str) -> np.ndarray:
    """Device result -> [rc, 64] node-major output for one core."""
    outT = np.asarray(res_map["outT"])
    if variant != "fp8pair":
        return np.ascontiguousarray(outT.T)
    # paired bf16 layout: outT[64*half + c, pair*512 + r] = node (2*pair+half)*512+r
    npair = NPAIR_F8
    main = (
        outT[:, : npair * R_TILE]
        .astype(np.float32)
        .reshape(2, F_OUT, npair, R_TILE)
        .transpose(2, 0, 3, 1)
        .reshape(npair * 2 * R_TILE, F_OUT)
    )
    tail = (
        outT[0:F_OUT, npair * R_TILE : npair * R_TILE + R_ODD]
        .astype(np.float32)
        .T
    )
    return np.ascontiguousarray(np.concatenate([main, tail], axis=0))


def _bern_alpha(theta: np.ndarray) -> np.ndarray:
    """Coefficients alpha_j of sum_k theta_k C(K,k)/2^K (1-t)^k (1+t)^{K-k}."""
    alpha = np.zeros(KBERN + 1, dtype=np.float64)
    for k in range(KBERN + 1):
        poly = np.array([1.0])
        for _ in range(k):
            poly = np.convolve(poly, [1.0, -1.0])  # (1 - t)
        for _ in range(KBERN - k):
            poly = np.convolve(poly, [1.0, 1.0])   # (1 + t)
        alpha += (comb(KBERN, k) / 2.0 ** KBERN) * float(theta[k]) * poly
    return alpha


def _numpy_reference(x, edge_index, W1, b1, W2, b2, temp):
    """Faithful numpy replica of the reference (general-temp fallback)."""
    n = x.shape[0]
    h = np.maximum(x @ W1 + b1, 0.0).astype(np.float32)
    h = (h @ W2 + b2).astype(np.float32)
    theta = np.maximum(temp.astype(np.float32), 0.0)
    row, col = edge_index[0], edge_index[1]
    deg = np.zeros(n, np.float32)
    np.add.at(deg, row, np.float32(1.0))
    dinv = np.where(deg > 0, 1.0 / np.sqrt(deg), 0.0).astype(np.float32)
    w = (dinv[row] * dinv[col])[:, None].astype(np.float32)

    def adj(v):
        out = np.zeros_like(v)
        np.add.at(out, row, v[col] * w)
        return out

    tmp = [h]
    v = h
    for _ in range(KBERN):
        v = v + adj(v)
        tmp.append(v)
    scale = np.float32(1.0 / 2.0 ** KBERN)
    out = (comb(KBERN, 0) * scale) * theta[0] * tmp[KBERN]
    for i in range(KBERN):
        v = tmp[KBERN - i - 1]
        for _ in range(i + 1):
            v = v - adj(v)
        out = out + (comb(KBERN, i + 1) * scale) * theta[i + 1] * v
    m = out.max(axis=1, keepdims=True)
    ex = np.exp(out - m)
    return ((out - m) - np.log(ex.sum(axis=1, keepdims=True))).astype(np.float32)


def kernel(**inputs) -> np.ndarray:
    x = np.asarray(inputs["x"], dtype=np.float32)
    W1 = np.ascontiguousarray(np.asarray(inputs["W1"], dtype=np.float32))
    b1 = np.ascontiguousarray(np.asarray(inputs["b1"], dtype=np.float32))
    W2 = np.ascontiguousarray(np.asarray(inputs["W2"], dtype=np.float32))
    b2 = np.ascontiguousarray(np.asarray(inputs["b2"], dtype=np.float32))
    temp = np.asarray(inputs["temp"], dtype=np.float32)
    edge_index = np.asarray(inputs["edge_index"])

    theta = np.maximum(temp.astype(np.float64), 0.0)
    alpha = _bern_alpha(theta)
    collapses = abs(alpha[0] - 1.0) < 1e-9 and np.all(np.abs(alpha[1:]) < 1e-9)
    if not (collapses and x.shape == (N_NODES, F_IN) and W1.shape == (F_IN, F_MID)
            and W2.shape == (F_MID, F_OUT)):
        return _numpy_reference(x, edge_index.astype(np.int64), W1, b1, W2, b2, temp)

    variant = _pick_variant(b1, b2)
    in_maps = _make_in_maps(x, W1, b1, W2, b2, variant)
    nc = _build_program(variant)
    res = run_bass_kernel_spmd(nc, in_maps, list(range(N_CORES))).results
    out = np.concatenate(
        [_gather_core(res[i], variant) for i in range(N_CORES)], axis=0
    )
    return np.ascontiguousarray(out[:N_NODES])

